# revision 64
# baseline (speedup 1.0000x reference)
"""Multi-head causal self-attention (B=2, T=2048, C=1024, H=16, D=64) on 8
Trainium2 NeuronCores.

Sharding: core = b*4 + g handles batch b and head group g (4 heads).
Each core computes QKV projection columns for its heads, full causal
attention for those heads, and the out-projection rows for those heads,
producing a partial [T, C] output. Host sums the 4 partials per batch and
adds b_proj.

All matmuls run in bf16 (vs the f32r predecessor): same 1 row/cycle PE
rate at large free dims, but no 4x penalty on short (free<256) diagonal
matmuls, 1.0 c/r transposes, FWL-accelerated weight loads, halved DMA
and 2x DVE throughput. Accumulation stays fp32 in PSUM; softmax
denominators/reciprocals in fp32. Measured end-to-end rel err ~5e-3
(tolerance 2e-2).

Key scheduling structure:
- S-score PSUM tiles are 2-bank pairs: both heads of a head-pair go in
  one [128,1024] tile, so one ACTIVATE does exp for both (halves the
  Act-engine instruction + semaphore count, which was co-critical).
- All big inputs are host-packed into [128, ...] fat-contiguous-line
  DRAM layouts (4-32KB per partition row) and each loads as ONE
  descriptor, balanced across the two HW DGE queues in deadline order
  (each ring holds only ~3 outstanding descriptors; thin 512B-row
  transfers crawl at ~35KB/us and are kept off the critical path).
  Biases/masks ride the HW queues packed+padded — gpsimd SW-DGE
  transfers throttled both HW queues' bandwidth to ~40%.
- A 30-matmul warmup spin on zeros bridges the PE from the framework
  preamble until block 0's inputs land (~16.5us, DMA-bound): it trips
  the HAM clock gate at ~11us and keeps it at 2.4 GHz so the real QKV
  stream never runs at the cold half-rate clock.
- S/exp run 2 tiles ahead of PV (software pipeline) so PV never waits
  on the Act engine's exp latency.
- QKV(g+1), V'(g+1) transposes and out-proj(g-1) are interleaved as
  filler work between attention tiles of block g; 2 units are reserved
  for the end of each block to cover the last normalize chain.
- Epilogue per head-pair: one [65,1024] PSUM->SBUF copy (outputs +
  denominator row) frees the PV accumulator banks in ~1.2us, so the
  next pair's PV(0) no longer waits for the full reciprocal/broadcast/
  normalize chain, which runs off-path from SBUF. (The custom-DVE
  reciprocal ignores the input AP's partition offset, so the
  denominator row is always rebased to a partition-0 tile first.)
- The last block's out-projection is split by contraction slab: the
  hp0 half runs during hp1's attention and ships via a second DRAM
  output (host adds it). The hp1 half runs as a per-128-q-chunk
  CASCADE: each diagonal PV tile finalizes one column chunk, whose
  denominator/reciprocal/broadcast/normalize and 128-row projection
  pipeline immediately, so the kernel tail is only the last chunk's
  short chain instead of a full-width epilogue + whole projection.
- PSUM budget: spair 3x2 banks (shared by S-pairs, QKV, transposes and
  out-proj psums) + PV-pair accumulator 2 = 8 banks.

- The tail cascade keeps the PE clock warm with dummy matmul spins and
  drains its projections entirely on the Act engine, leaving the DVE
  free for the per-chunk reciprocal/normalize chain.

Measured: 164.4us (session baseline 172.6us; first f32r kernel
203.0us), rel err 5.5e-3.

Softmax skips the row-max subtraction: scaled scores are bounded by ~8,
so exp() stays finite in fp32/bf16.
"""
import sys

if '/opt/trn_rl_repo' not in sys.path:
    sys.path.insert(0, '/opt/trn_rl_repo')

import os
import numpy as np
import ml_dtypes

import concourse.bass as bass
import concourse.bacc as bacc
import concourse.mybir as mybir
import concourse.tile as tile
from concourse.bass_utils import run_bass_kernel_spmd
from concourse.masks import make_identity

f32 = mybir.dt.float32
bf16 = mybir.dt.bfloat16
AFT = mybir.ActivationFunctionType

B, T, C = 2, 2048, 1024
H, D = 16, 64
HPC = 4                 # heads per core
GC = HPC * D            # columns per core in qkv space (256)
N_CORES = 8
QB = 512                # q block (free dim of S^T tiles)
KT = 128                # k tile (partition dim of S^T tiles)
NQB = T // QB           # 4
VW = 68                 # padded stride of per-(ktile,head) V' block (65 used)
NM = GC // 128          # 2 head-pair slabs
NCT = C // 128          # 8 contraction tiles


def _bf16(a: np.ndarray) -> np.ndarray:
    return np.ascontiguousarray(a, np.float32).astype(ml_dtypes.bfloat16)


def _pack(a: np.ndarray) -> np.ndarray:
    """[K*128, N] -> [128, K*N] so dest[p, k, n] = a[k*128+p, n]: every
    partition's DRAM line is contiguous (fat-line single-descriptor DMA)."""
    k = a.shape[0] // 128
    return np.ascontiguousarray(
        np.asarray(a).reshape(k, 128, -1).transpose(1, 0, 2).reshape(128, -1))


def _build():
    nc = bacc.Bacc(None, target_bir_lowering=False, debug=False)

    # All big inputs are pre-packed on the host into [128, ...] row-major
    # layouts whose per-partition DRAM lines are fat and contiguous
    # (4-32KB), so each input is 1 descriptor at full HBM throughput
    # instead of many thin-line chunks (512B-1KB lines ran at ~half rate
    # and 0.6us descriptor-issue each serialized the startup).
    xt = nc.declare_dram_parameter("xt", [128, NCT * T], bf16, isOutput=False)
    wq = nc.declare_dram_parameter("wq", [128, NCT * GC], bf16, isOutput=False)
    wk = nc.declare_dram_parameter("wk", [128, NCT * GC], bf16, isOutput=False)
    wv = nc.declare_dram_parameter("wv", [128, NCT * GC], bf16, isOutput=False)
    # biases packed [128, 128] f32 (bq|bk|bv in cols 0:6, rest pad — thin
    # 24B rows transfer pathologically slowly) and mask doubled
    # [128, 2*KT]: tiny HW-queue transfers instead of gpsimd SW-DGE DMAs,
    # whose descriptor processing was throttling both HW queues' bandwidth
    # to ~40% exactly while block 0's inputs were in flight
    bqkv = nc.declare_dram_parameter("bqkv", [128, 128], f32, isOutput=False)
    wp = nc.declare_dram_parameter("wp", [128, NM * C], bf16, isOutput=False)
    msk = nc.declare_dram_parameter("msk", [128, 2 * KT], bf16, isOutput=False)
    out = nc.declare_dram_parameter("out", [T, C], bf16, isOutput=True)
    # hp0's share of the LAST block's projection, shipped separately and
    # summed on the host: removes the on-device combine from the tail
    out2 = nc.declare_dram_parameter("out2", [QB, C], bf16, isOutput=True)

    with tile.TileContext(nc) as tc:
        with tc.tile_pool(name="consts", bufs=1) as consts, \
             tc.tile_pool(name="stage", bufs=2) as stage, \
             tc.tile_pool(name="big", bufs=1) as big, \
             tc.tile_pool(name="epool", bufs=5) as epool, \
             tc.tile_pool(name="lpool", bufs=2) as lpool, \
             tc.tile_pool(name="pvsp", bufs=2) as pvsp, \
             tc.tile_pool(name="sp", bufs=3, space="PSUM") as sp, \
             tc.tile_pool(name="pp", bufs=1, space="PSUM") as pp:

            # ---- warmup feed tiles first: nothing may delay the PE spin ----
            wz = consts.tile([128, 128], bf16)
            nc.vector.memset(wz, 0.0)
            wzm = consts.tile([128, QB], bf16)
            nc.vector.memset(wzm, 0.0)

            # ---- warmup: trip the HAM clock gate while DMAs land ----
            # 30 N=512 matmuls bridge the PE from ~7.7us (first issue)
            # until block 0's inputs land (~16.4us, DMA-bound and very
            # consistent): ~8 run at the cold 1.2 GHz clock, the HAM
            # SHORT window then unthrottles to 2.4 GHz and the rest keep
            # the clock warm so the real QKV stream runs at full rate
            # from its first matmul instead of half-rate for 4.4us.
            # Uses the pv-pair PSUM tile (idle until block 0's attention).
            wps = pp.tile([128, 1024], f32, tag="pv", name="warm")
            for _w in range(30):
                nc.tensor.matmul(wps[:, 0:QB], wz, wzm,
                                 start=True, stop=True, skip_group_check=True)

            # ---- constants ----
            ident = consts.tile([128, 128], f32)
            make_identity(nc, ident)
            identb = consts.tile([128, 128], bf16)
            nc.vector.tensor_copy(identb, ident)
            onesb = consts.tile([128, 1], bf16)
            nc.vector.memset(onesb, 1.0)
            bqkv_sb = consts.tile([128, 128], f32)
            bq_sb = bqkv_sb[:, 0:2]
            bk_sb = bqkv_sb[:, 2:4]
            bv_sb = bqkv_sb[:, 4:6]
            mskd = consts.tile([128, 2, KT], bf16)

            # ---- persistent tiles ----
            # x^T resident per q-block: host packs x block-major so each
            # block's [128, 8ct, 512] region is ONE fully-contiguous
            # 8KB-line descriptor (512KB). Block 0 lands ~2us after issue,
            # so QKV(0) streams immediately; separate tiles keep the first
            # matmul's dependency on block 0's DMA only.
            xsb = [big.tile([128, NCT, QB], bf16, tag=f"xT{g}", name=f"xT{g}")
                   for g in range(NQB)]
            ktq = [[big.tile([128, QB], bf16, tag=f"kt{m}_{g}", name=f"kt{m}_{g}")
                    for g in range(NQB)] for m in range(NM)]
            vtq = [[big.tile([128, QB], bf16, tag=f"vyt{m}_{g}", name=f"vt{m}_{g}")
                    for g in range(NQB)] for m in range(NM)]
            # q pair-tiles [128, 2hh, QB]: the two heads of a pair sit in
            # adjacent column groups so ONE matmul (moving free 2*QB bf16)
            # computes both heads' scores against the shared K stationary
            qthp = [[big.tile([128, 2, QB], bf16, tag=f"qth{m}_{g}",
                              name=f"qth{m}_{g}")
                     for g in range(NQB)] for m in range(NM)]
            # flat whole-tile zero memsets (the bias-add later overwrites
            # the data halves); strided half-memsets were 3x slower on DVE
            for m in range(NM):
                for g in range(NQB):
                    nc.vector.memset(qthp[m][g], 0.0)

            wq_sb = big.tile([128, NCT, GC], bf16, tag="wq")
            wk_sb = big.tile([128, NCT, GC], bf16, tag="wk")
            wv_sb = big.tile([128, NCT, GC], bf16, tag="wv")
            wp_sb = big.tile([128, NM, C], bf16, tag="wp")

            # ---- input DMAs ----
            # Every transfer is a single contiguous-line descriptor. The
            # two HW queues share HBM bandwidth (~200-250 GB/s each when
            # both active) and each DGE ring only holds ~3 outstanding
            # descriptors, so the load is BALANCED across queues in
            # deadline order. Thin-row transfers (bqkv, mskd: 512B rows,
            # ~35KB/us!) are placed where their slowness is off-path.
            XB = NCT * QB
            nc.sync.dma_start(out=wk_sb, in_=wk[:, :])
            nc.scalar.dma_start(out=bqkv_sb, in_=bqkv[:, :])
            nc.sync.dma_start(out=xsb[0], in_=xt[:, 0 * XB:1 * XB])
            nc.scalar.dma_start(out=wv_sb, in_=wv[:, :])
            nc.sync.dma_start(out=wq_sb, in_=wq[:, :])
            nc.scalar.dma_start(out=xsb[2], in_=xt[:, 2 * XB:3 * XB])
            nc.sync.dma_start(out=xsb[1], in_=xt[:, 1 * XB:2 * XB])
            nc.scalar.dma_start(out=mskd, in_=msk[:, :])
            nc.scalar.dma_start(out=xsb[3], in_=xt[:, 3 * XB:4 * XB])
            nc.scalar.dma_start(out=wp_sb, in_=wp[:, :])

            vpg = [None] * NQB   # V' natural-layout tiles, built per block
            ytq = [[None] * NQB for _ in range(NM)]

            wmap = {"q": (wq_sb, bq_sb), "k": (wk_sb, bk_sb), "v": (wv_sb, bv_sb)}

            def qkv_bias(g, kind, m, ph, eng=None):
                b_sb = wmap[kind][1]
                eng = eng or nc.vector
                if kind == "q":
                    for hh in range(2):
                        o = 64 * hh
                        eng.tensor_scalar_add(
                            qthp[m][g][o:o + 64, hh, :],
                            ph[o:o + 64, :], b_sb[o:o + 64, m:m + 1])
                else:
                    dest = (ktq if kind == "k" else vtq)[m][g]
                    eng.tensor_scalar_add(dest, ph, b_sb[:, m:m + 1])

            def qkv_group(g, kind, m):
                w_sb, _ = wmap[kind]
                ppt = sp.tile([128, 1024], f32, tag="spair", name=f"pp_{g}{kind}{m}")
                ph = ppt[:, 0:512]
                for ct in range(NCT):
                    nc.tensor.matmul(
                        ph, w_sb[:, ct, m * 128:(m + 1) * 128], xsb[g][:, ct, :],
                        start=(ct == 0), stop=(ct == NCT - 1),
                        skip_group_check=True)
                qkv_bias(g, kind, m, ph)

            def vprime_unit(g, m):
                # V' natural-layout V + ones column: 16 blocks of VW cols,
                # col 64 = 1.0 (emits the softmax denominator as PSUM row 64
                # of the PV matmul). The PV stationary over-reads 128 cols
                # from each block start; junk lands in unread PSUM rows.
                if vpg[g] is None:
                    vp = big.tile([128, 4 * HPC * VW + 128], bf16,
                                  tag=f"vp{g}", name=f"vp{g}")
                    vpg[g] = vp
                    vpv = vp[:, 0:4 * HPC * VW].rearrange("p (b w) -> p b w", w=VW)
                    nc.vector.tensor_copy(
                        vpv[:, 0:4 * HPC, 64:65],
                        onesb.to_broadcast([128, 4 * HPC, 1]))
                vp = vpg[g]
                ptile = sp.tile([128, 1024], f32, tag="spair", name=f"vt_{g}{m}")
                ptb = ptile.bitcast(bf16)
                for lt in range(4):
                    nc.tensor.transpose(
                        ptb[:, lt * 128:(lt + 1) * 128],
                        vtq[m][g][:, lt * 128:(lt + 1) * 128], identb)
                src = ptb[:, 0:512].rearrange("p (l h d) -> p l h d", l=4, h=2)
                vpv4 = vp[:, 0:4 * HPC * VW].rearrange(
                    "p (l h w) -> p l h w", l=4, h=HPC)
                nc.vector.tensor_copy(vpv4[:, :, 2 * m:2 * m + 2, 0:64], src)

            def proj_unit(g, lt):
                tt = 4 * g + lt
                po = sp.tile([128, 1024], f32, tag="spair", name=f"po{tt}")
                for n in range(2):
                    for m in range(NM):
                        nc.tensor.matmul(
                            po[:, n * 512:(n + 1) * 512],
                            ytq[m][g][:, lt * 128:(lt + 1) * 128],
                            wp_sb[:, m, n * 512:(n + 1) * 512],
                            start=(m == 0), stop=(m == NM - 1),
                            skip_group_check=True)
                ot = stage.tile([128, C], bf16, tag="stage", name=f"ot{tt}")
                # split PSUM->SBUF drains between DVE and Act; lt 2/3 are the
                # units reserved for block boundaries, where their copy must
                # NOT sit in the DVE queue ahead of the epilogue's
                # reciprocal/normalize chain
                if lt < 2:
                    nc.vector.tensor_copy(ot, po)
                else:
                    nc.scalar.activation(ot, po, AFT.Copy)
                nc.sync.dma_start(out=out[tt * 128:(tt + 1) * 128, :], in_=ot)

            # the last block's projection is split by contraction slab: the
            # hp0 half runs during hp1's attention and ships via out2 (the
            # host adds it), so the tail is just the hp1 half.
            def proj_m0_unit(g, lt):
                tt = 4 * g + lt
                po = sp.tile([128, 1024], f32, tag="spair", name=f"poa{tt}")
                for n in range(2):
                    nc.tensor.matmul(
                        po[:, n * 512:(n + 1) * 512],
                        ytq[0][g][:, lt * 128:(lt + 1) * 128],
                        wp_sb[:, 0, n * 512:(n + 1) * 512],
                        start=True, stop=True, skip_group_check=True)
                pt = stage.tile([128, C], bf16, tag="pst", name=f"pst{tt}")
                if lt % 2 == 0:
                    nc.scalar.activation(pt, po, AFT.Copy)
                else:
                    nc.vector.tensor_copy(pt, po)
                nc.sync.dma_start(out=out2[lt * 128:(lt + 1) * 128, :], in_=pt)

            def proj_m1_unit(g, lt):
                # tail-only: drain entirely on the Act engine — the DVE
                # carries the cascade's lr/reciprocal/normalize chain and
                # must not be blocked by 0.7us PSUM-copy halves
                tt = 4 * g + lt
                po = sp.tile([128, 1024], f32, tag="spair", name=f"pob{tt}")
                for n in range(2):
                    nc.tensor.matmul(
                        po[:, n * 512:(n + 1) * 512],
                        ytq[1][g][:, lt * 128:(lt + 1) * 128],
                        wp_sb[:, 1, n * 512:(n + 1) * 512],
                        start=True, stop=True, skip_group_check=True)
                ot = stage.tile([128, C], bf16, tag="stage", name=f"ot{tt}")
                if lt == 3:
                    # final tile: the DVE is idle by now, so split the
                    # drain across both engines to halve the last chain's
                    # PSUM->SBUF latency
                    nc.scalar.activation(ot[:, 0:512], po[:, 0:512],
                                         AFT.Copy)
                    nc.vector.tensor_copy(ot[:, 512:1024], po[:, 512:1024])
                else:
                    nc.scalar.activation(ot, po, AFT.Copy)
                if lt % 2 == 0:
                    nc.scalar.dma_start(out=out[tt * 128:(tt + 1) * 128, :], in_=ot)
                else:
                    nc.sync.dma_start(out=out[tt * 128:(tt + 1) * 128, :], in_=ot)

            # prologue: QKV + V' of block 0, kind order matching weight
            # arrival (wk, wv, wq) — x block 0 is fully resident by then
            for kind in ("k", "v", "q"):
                for m in range(NM):
                    qkv_group(0, kind, m)
            for m in range(NM):
                vprime_unit(0, m)

            for g in range(NQB):
                # filler units interleaved into this block's attention,
                # ordered so consumers of a unit's DVE drain (e.g. V'
                # transposes after the v-projection) aren't adjacent to it
                prj = [(proj_unit, (g - 1, lt)) for lt in range(4)]
                qkv = [(qkv_group, (g + 1, kind, m))
                       for kind in ("q", "k", "v") for m in range(NM)]
                vpu = [(vprime_unit, (g + 1, m)) for m in range(NM)]
                if 0 < g < NQB - 1:
                    # QKV(g+1) early (next block's start depends on it),
                    # proj(g-1) late (fills the Act-bound back half)
                    q0, q1, k0, k1, v0, v1 = qkv
                    units = [q0, q1, k0, k1, v0, prj[0], v1, prj[1],
                             vpu[0], prj[2], vpu[1], prj[3]]
                elif g == 0:
                    units = qkv + vpu
                else:
                    units = prj

                nkt = 4 * g + 4
                ntiles = NM * nkt
                # reserve units for the end of the block to cover the last
                # epilogue chain; the final block spreads everything (its
                # tail is covered by the per-chunk cascade)
                nspread = max(len(units) - (0 if g == NQB - 1 else 2), 0)
                ui = 0
                tj = 0
                LA = 2                  # S/exp run this many tiles ahead of PV
                for hp in range(NM):
                    ytq[hp][g] = big.tile([128, QB], bf16, tag=f"vyt{hp}_{g}",
                                          name=f"yt{hp}_{g}")
                    pv = pp.tile([128, 1024], f32, tag="pv", name=f"pv{g}_{hp}")
                    pvv = pv.rearrange("p (b q) -> p b q", b=2)
                    eps = {}
                    lbs = {}
                    cascade = g == NQB - 1 and hp == 1

                    def tail_recip(r, pvv=pvv):
                        # 2 dummy warm-keeper matmuls (into a rotating
                        # spair slot) fill the PE pipeline while this
                        # chunk's epilogue chain runs, so the HAM MID
                        # window never re-throttles the clock and the
                        # tail projections run at full rate
                        wk_ps = sp.tile([128, 1024], f32, tag="spair",
                                        name=f"wk{r}")
                        for _w in range(2):
                            nc.tensor.matmul(
                                wk_ps[:, 0:QB], wz, wzm,
                                start=True, stop=True,
                                skip_group_check=True)
                        # denominator chunk r (final after diagonal PV
                        # tile 4g+r): rebase to a partition-0 tile first —
                        # the custom-DVE reciprocal ignores the input AP's
                        # partition offset, so it must never read PSUM row
                        # 64 directly
                        cs = slice(r * KT, (r + 1) * KT)
                        lr = lpool.tile([1, 2, KT], f32, tag="tlr",
                                        name=f"tlr{r}")
                        for hh in range(2):
                            nc.vector.tensor_copy(lr[:, hh, :],
                                                  pvv[64:65, hh, cs])
                        linv = lpool.tile([1, 2, KT], f32, tag="tl",
                                          name=f"tl{r}")
                        nc.vector.reciprocal_approx_fast(out=linv, in_=lr)
                        linv_b = lpool.tile([64, 2, KT], f32, tag="tlb",
                                            name=f"tlb{r}")
                        nc.gpsimd.partition_broadcast(linv_b, linv)
                        lbs[r] = linv_b

                    def tail_finish(r, pvv=pvv, g=g):
                        # normalize q-chunk r straight out of PSUM and run
                        # its 128-row out-projection immediately
                        cs = slice(r * KT, (r + 1) * KT)
                        for hh in range(2):
                            off = 64 * hh
                            nc.vector.tensor_mul(
                                ytq[1][g][off:off + 64, cs],
                                pvv[0:64, hh, cs], lbs[r][:, hh, :])
                        proj_m1_unit(g, r)

                    def emit_S(i, hp=hp, eps=eps):
                        r = i - 4 * g
                        lo = max(r, 0) * 128
                        spt = sp.tile([128, 1024], f32, tag="spair",
                                      name=f"s{g}_{hp}_{i}")
                        spv = spt.rearrange("p (b q) -> p b q", b=2)
                        ks = (i % 4) * 128
                        for hh in range(2):  # share the kt-slice stationary
                            nc.tensor.matmul(
                                spv[:, hh, lo:QB],
                                ktq[hp][i // 4][:, ks:ks + 128],
                                qthp[hp][g][:, hh, lo:QB],
                                start=True, stop=True, skip_group_check=True)
                        ep = epool.tile([128, 2, QB], bf16, tag="e",
                                        name=f"e{g}_{hp}_{i}")
                        nc.scalar.activation(ep[:, :, lo:QB], spv[:, :, lo:QB],
                                             AFT.Exp, scale=0.125)
                        if r >= 0:
                            nc.vector.tensor_mul(
                                ep[:, :, lo:lo + KT], ep[:, :, lo:lo + KT], mskd)
                        eps[i] = ep

                    def emit_PV(i, hp=hp, pv=pv, eps=eps):
                        r = i - 4 * g
                        lo = max(r, 0) * 128
                        ep = eps.pop(i)
                        for hh in range(2):
                            blk = ((i % 4) * HPC + 2 * hp + hh) * VW
                            nc.tensor.matmul(
                                pv[:, hh * 512 + lo:hh * 512 + QB],
                                vpg[i // 4][:, blk:blk + 128],
                                ep[:, hh, lo:QB],
                                start=(i == 0), stop=(i == nkt - 1),
                                skip_group_check=True)

                    for i in range(nkt + LA):
                        if i < nkt:
                            emit_S(i)
                        j = i - LA
                        if j < 0:
                            continue
                        # filler goes between S(i) and PV(j): it keeps the PE
                        # busy while the Act engine's exp(j) drains, instead
                        # of PV(j) stalling on it
                        target = min(nspread, tj * nspread // max(1, ntiles - 2))
                        if j == 0:
                            # hp/block boundary: PV(0) waits for the previous
                            # epilogue chain to free the accumulator — cover it
                            target = min(nspread, target + 3)
                        while ui < target:
                            fn, args = units[ui]
                            fn(*args)
                            ui += 1
                        emit_PV(j)
                        tj += 1
                        if g == NQB - 1 and hp == 1 and j in (6, 8, 10, 12):
                            proj_m0_unit(g, j // 2 - 3)
                        if cascade:
                            if j in (10, 11):
                                # pre-cascade warm-keepers: the tail end
                                # of hp1's attention is exp-gated with no
                                # filler units left, and this idle window
                                # is what used to trip the HAM re-throttle
                                wk_ps = sp.tile([128, 1024], f32,
                                                tag="spair", name=f"wkp{j}")
                                for _w in range(2):
                                    nc.tensor.matmul(
                                        wk_ps[:, 0:QB], wz, wzm,
                                        start=True, stop=True,
                                        skip_group_check=True)
                            r = j - 4 * g
                            if r >= 0:
                                tail_recip(r)
                                if r >= 1:
                                    tail_finish(r - 1)
                    if cascade:
                        tail_finish(3)
                        continue
                    # epilogue for both heads: the denominator-row copy
                    # plus one [64,1024] copy free the PV accumulator
                    # banks in ~2.4us of DVE work, so the next head-pair's
                    # PV(0) no longer waits for the full reciprocal/
                    # broadcast/normalize chain — that runs off-path from
                    # SBUF while the next pair's attention streams
                    pvs = pvsp.tile([65, 2 * QB], f32, tag="pvs",
                                    name=f"pvs{g}_{hp}")
                    nc.vector.tensor_copy(pvs[:, 0:QB], pv[0:65, 0:QB])
                    nc.vector.tensor_copy(pvs[:, QB:2 * QB],
                                          pv[0:65, QB:2 * QB])
                    # rebase the denominator row to partition 0 (plain
                    # copy handles the offset; the custom-DVE reciprocal
                    # does not) — off the bank-release critical path
                    lrow = lpool.tile([1, 2 * QB], f32, tag="lr")
                    nc.vector.tensor_copy(lrow, pvs[64:65, :])
                    linv = lpool.tile([1, 2 * QB], f32, tag="l")
                    nc.vector.reciprocal_approx_fast(out=linv, in_=lrow)
                    linv_b = lpool.tile([64, 2 * QB], f32, tag="lb")
                    nc.gpsimd.partition_broadcast(linv_b, linv)
                    for hh in range(2):
                        off = 64 * hh
                        nc.vector.tensor_mul(
                            ytq[hp][g][off:off + 64, :],
                            pvs[0:64, hh * 512:(hh + 1) * 512],
                            linv_b[:, hh * 512:(hh + 1) * 512])
                while ui < len(units):
                    fn, args = units[ui]
                    fn(*args)
                    ui += 1

            # (the last block's hp1 projection already ran in the cascade)

    nc.finalize()
    return nc


_NC = None


def _get_nc():
    global _NC
    if _NC is None:
        _NC = _build()
    return _NC


_LAST_RESULTS = None  # BassKernelResults of the most recent run (for test.py)


def kernel(x, W_qkv, b_qkv, W_proj, b_proj):
    x = np.ascontiguousarray(np.asarray(x), dtype=np.float32)
    W_qkv = np.asarray(W_qkv, dtype=np.float32)
    b_qkv = np.asarray(b_qkv, dtype=np.float32)
    W_proj = np.asarray(W_proj, dtype=np.float32)
    b_proj = np.asarray(b_proj, dtype=np.float32)

    # in-tile causal mask for diagonal S^T tiles: valid iff local q col >= p
    masks = (np.arange(KT)[None, :] >= np.arange(KT)[:, None]).astype(np.float32)

    in_maps = []
    for core in range(N_CORES):
        b, g = divmod(core, 4)
        cs = slice(g * GC, (g + 1) * GC)
        xT = np.ascontiguousarray(x[b].T)
        xtp = np.concatenate([_pack(xT[:, g * QB:(g + 1) * QB])
                              for g in range(NQB)], axis=1)
        bqkv = np.zeros((128, 128), dtype=np.float32)
        bqkv[:, 0:6] = np.concatenate(
            [b_qkv[k * C:k * C + C][cs].reshape(2, 128).T for k in range(3)],
            axis=1)
        in_maps.append({
            "xt": _bf16(xtp),
            "wq": _bf16(_pack(W_qkv[:, 0 * C:1 * C][:, cs])),
            "wk": _bf16(_pack(W_qkv[:, 1 * C:2 * C][:, cs])),
            "wv": _bf16(_pack(W_qkv[:, 2 * C:3 * C][:, cs])),
            "bqkv": np.ascontiguousarray(bqkv),
            "wp": _bf16(_pack(W_proj[cs, :])),
            "msk": _bf16(np.concatenate([masks, masks], axis=1)),
        })

    nc = _get_nc()
    trace = os.environ.get("BASSKERNEL_TRACE", "0") == "1"
    res = run_bass_kernel_spmd(nc, in_maps, core_ids=list(range(N_CORES)),
                               trace=trace)
    global _LAST_RESULTS
    _LAST_RESULTS = res

    partials = np.stack([np.asarray(res.results[i]["out"], dtype=np.float64)
                         for i in range(N_CORES)])
    partials = partials.reshape(B, 4, T, C)
    out = partials.sum(axis=1) + b_proj.astype(np.float64)
    # hp0's share of the last q block's projection travels via out2
    p2 = np.stack([np.asarray(res.results[i]["out2"], dtype=np.float64)
                   for i in range(N_CORES)])
    out[:, T - QB:T, :] += p2.reshape(B, 4, QB, C).sum(axis=1)
    return out.astype(np.float32)

